# revision 16
# baseline (speedup 1.0000x reference)
"""GAT (layer-0 only — reference discards layers 1/2) + MLP head on 8 trn2 NeuronCores.

Nodes are sharded across 8 cores by destination. Because the fast gather engine
(InstDMAGatherAnt) takes int16 indices, the node-feature table is addressed
through 4 source-range windows of ~25k rows. Each range pass uses its own
degree-sorted tiling of the core's destination nodes (padding ~5%), produces
per-range partial [msg_sum|den] rows in HBM, and the partials are recombined by
permutation gathers. Final global centering/scale via two tiny AllReduces.
"""
import sys

sys.path.insert(0, "/opt/trn_rl_repo")

import math

import numpy as np
import ml_dtypes

import concourse.bacc as bacc
import concourse.bass as bass
import concourse.bass_isa as bass_isa
import concourse.mybir as mybir
import concourse.tile as tile
from concourse.bass_utils import run_bass_kernel_spmd
from concourse.masks import make_identity

F32 = mybir.dt.float32
BF16 = mybir.dt.bfloat16
I16 = mybir.dt.int16
AF = mybir.ActivationFunctionType
ALU = mybir.AluOpType

NEG_SLOPE = 0.2
HEADS = 8
HD = 8
HID = 64
D_IN = 128
NRANGE = 4
SENT_ALS = -200.0
TWO_PI = 2.0 * math.pi
RND_MAGIC = 12582912.0  # 1.5 * 2**23, float32 round-to-nearest trick
GROW = 128  # bf16 elements per gtab row (256 B)
MAX_IDX_PER_CALL = 12288

FULL = dict(N=100000, NC=8)


def _derive(N, NC):
    npc = N // NC
    T = (npc + 127) // 128
    rng_sz = (N + NRANGE - 1) // NRANGE  # nodes per range window
    win = rng_sz + 1  # +1 sentinel row per window
    nrows = NRANGE * win
    nch = (nrows + 127) // 128
    return npc, T, rng_sz, win, nch


def _wrap16(flat):
    """int16 idx list -> [128, ceil(n/16)] wrapped/replicated layout."""
    n = len(flat)
    nc16 = (n + 15) // 16
    a = np.zeros(nc16 * 16, np.int16)
    a[:n] = flat
    return np.ascontiguousarray(np.tile(a.reshape(nc16, 16).T, (8, 1)))


# ---------------------------------------------------------------- host prep
def _host_prep(x, edge_index, N, NC):
    npc, T, rng_sz, win, nch = _derive(N, NC)
    x = np.asarray(x, np.float32)
    src = np.concatenate([np.asarray(edge_index[0]), np.arange(N)]).astype(np.int64)
    dst = np.concatenate([np.asarray(edge_index[1]), np.arange(N)]).astype(np.int64)
    erange = src // rng_sz

    # per-(core,range) degree-sorted permutations and tile K's
    perms = [[None] * NRANGE for _ in range(NC)]
    slot_of = [[None] * NRANGE for _ in range(NC)]
    Kt = np.zeros((NRANGE, T), np.int64)
    deg_r = np.zeros((NRANGE, N), np.int64)
    for r in range(NRANGE):
        m = erange == r
        deg_r[r] = np.bincount(dst[m], minlength=N)
    for c in range(NC):
        blk = np.arange(c * npc, (c + 1) * npc)
        for r in range(NRANGE):
            order = np.argsort(-deg_r[r][blk], kind="stable")
            perm = blk[order]
            perms[c][r] = perm
            so = np.empty(npc, np.int64)
            so[order] = np.arange(npc)
            slot_of[c][r] = so  # local node offset -> slot
            dpad = np.zeros(T * 128, np.int64)
            dpad[:npc] = deg_r[r][perm]
            Kt[r] = np.maximum(Kt[r], dpad.reshape(T, 128).max(1))
    Kt = np.maximum(Kt, 1)

    # group tiles into gather calls; tiles in a group share K = max (tiles are
    # degree-sorted so consecutive K are close). Each group: (t0, ntiles, Kg).
    groups = [[] for _ in range(NRANGE)]
    for r in range(NRANGE):
        t = 0
        while t < T:
            Kg = int(Kt[r][t])
            gmax = max(1, min(7, MAX_IDX_PER_CALL // (128 * Kg), 96 // Kg if Kg <= 96 else 1))
            gn = min(gmax, T - t)
            groups[r].append((t, gn, Kg))
            t += gn

    # per-core edge index arrays (wrapped int16, concatenated per range)
    idx_arrays = []
    rel_arrays = []
    xtloc_arrays = []
    for c in range(NC):
        per_range = []
        for r in range(NRANGE):
            m = (erange == r) & (dst // npc == c)
            s = slot_of[c][r][dst[m] - c * npc]
            sr = (src[m] - r * rng_sz).astype(np.int64)  # local row in window
            o = np.argsort(s, kind="stable")
            s = s[o]
            sr = sr[o]
            starts = np.searchsorted(s, np.arange(npc + 1))
            k = np.arange(len(s)) - starts[np.minimum(s, npc - 1)]
            wraps = []
            for (t0, gn, Kg) in groups[r]:
                A = np.full((gn, Kg, 128), rng_sz, np.int64)
                for g in range(gn):
                    t = t0 + g
                    lo, hi = starts[t * 128], starts[min((t + 1) * 128, npc)]
                    A[g, k[lo:hi], s[lo:hi] - t * 128] = sr[lo:hi]
                wraps.append(_wrap16(A.reshape(-1)))
            per_range.append(np.concatenate(wraps, axis=1))
        idx_arrays.append(per_range)
        # relative permutations for combining partials (into range-0 order)
        rels = []
        for r in range(1, NRANGE):
            rel = np.zeros(T * 128, np.int64)
            rel[: npc] = slot_of[c][r][perms[c][0] - c * npc]
            rels.append(_wrap16(rel))
        rel_arrays.append(np.concatenate(rels, axis=1))
        # per-range local x.T slices for the al_d mini-pass (bf16)
        xl = np.zeros((NRANGE, D_IN, T * 128), np.float32)
        for r in range(NRANGE):
            xl[r][:, :npc] = x[perms[c][r]].T
        xtloc_arrays.append(
            np.ascontiguousarray(
                xl.reshape(NRANGE, D_IN, T, 128).transpose(0, 2, 1, 3)
            )
            .astype(ml_dtypes.bfloat16)
            .reshape(NRANGE * T * D_IN, 128)
        )

    mk = np.zeros(T * 128, np.float32)
    mk[:npc] = 1.0
    maskt = np.ascontiguousarray(mk.reshape(T, 128).T)

    # x.T in gtab-row order (sentinel/pad columns zero), chunked [nch,128,128]
    xtp = np.zeros((D_IN, nch * 128), np.float32)
    rows = np.arange(NRANGE * win)
    node_of_row = rows // win * rng_sz + rows % win  # sentinel rows map to...
    valid = (rows % win < rng_sz) & (node_of_row < N)
    xtp[:, rows[valid]] = x[node_of_row[valid]].T
    xtc = (
        np.ascontiguousarray(xtp.reshape(D_IN, nch, 128).transpose(1, 0, 2))
        .astype(ml_dtypes.bfloat16)
        .reshape(nch * 128, 128)
    )

    return dict(
        perms=perms, Kt=Kt, groups=groups, kofs=None, idx=idx_arrays,
        rel=rel_arrays, xtloc=xtloc_arrays, maskt=maskt, xtc=xtc,
        npc=npc, T=T, rng_sz=rng_sz, win=win, nch=nch,
    )


# ---------------------------------------------------------------- program
def _build_program(N, NC, Kt, groups, kofs, reduce_mode="dve", timing=False, phases="gaect"):
    npc, T, rng_sz, win, nch = _derive(N, NC)
    NT = T * 128

    nc = bacc.Bacc("TRN2", target_bir_lowering=False, debug=False, num_devices=(1 if timing else NC))

    idx_cols = [
        sum((gn * Kg * 128) // 16 for (_, gn, Kg) in groups[r])
        for r in range(NRANGE)
    ]
    xt = nc.dram_tensor("xt", [nch * 128, D_IN], BF16, kind="ExternalInput")
    xtloc = nc.dram_tensor("xtloc", [NRANGE * T * D_IN, 128], BF16, kind="ExternalInput")
    wcat_d = nc.dram_tensor("wcat", [D_IN, 80], F32, kind="ExternalInput")
    idx_d = [
        nc.dram_tensor(f"idx{r}", [128, idx_cols[r]], I16, kind="ExternalInput")
        for r in range(NRANGE)
    ]
    rel_d = nc.dram_tensor(
        "rel", [128, (NRANGE - 1) * ((NT + 15) // 16)], I16, kind="ExternalInput"
    )
    maskt_d = nc.dram_tensor("maskt", [128, T], F32, kind="ExternalInput")
    awcat_d = nc.dram_tensor("awcat", [HID, 96], F32, kind="ExternalInput")
    aw2_d = nc.dram_tensor("aw2r", [128, HID], F32, kind="ExternalInput")
    rw2_d = nc.dram_tensor("rw2r", [128, 32], F32, kind="ExternalInput")

    gtab = nc.dram_tensor("gtab", [nch * 128, GROW], BF16)
    partials = [
        nc.dram_tensor(f"part{r}", [NT, 128], F32) for r in range(1, NRANGE)
    ]
    ar1i = nc.dram_tensor("ar1i", [1, 2], F32)
    ar1o = nc.dram_tensor("ar1o", [1, 2], F32, addr_space="Shared")
    ar2i = nc.dram_tensor("ar2i", [1, 1], F32)
    ar2o = nc.dram_tensor("ar2o", [1, 1], F32, addr_space="Shared")
    coords = nc.dram_tensor("coords", [NT, 2], F32, kind="ExternalOutput")

    rg = [list(range(NC))]

    with tile.TileContext(nc) as tc:
        with tc.tile_pool(name="persist", bufs=1) as persist:
            wcat_f = persist.tile([D_IN, 80], F32)
            nc.sync.dma_start(wcat_f[:], wcat_d.ap())
            wcat = persist.tile([D_IN, 80], BF16)
            nc.vector.tensor_copy(wcat[:], wcat_f[:])
            identf = persist.tile([128, 128], F32)
            make_identity(nc, identf[:])
            awcat = persist.tile([HID, 96], F32)
            nc.sync.dma_start(awcat[:], awcat_d.ap())
            aw2r = persist.tile([128, HID], F32)
            nc.sync.dma_start(aw2r[:], aw2_d.ap())
            rw2r = persist.tile([128, 32], F32)
            nc.sync.dma_start(rw2r[:], rw2_d.ap())
            maskt = persist.tile([128, T], F32)
            nc.sync.dma_start(maskt[:], maskt_d.ap())
            eps_c = persist.tile([128, 1], F32)
            nc.vector.memset(eps_c[:], 1e-5)
            b09_c = persist.tile([128, 1], F32)
            nc.vector.memset(b09_c[:], 0.9)

            # per-range al_d for local dst nodes [128, T, 8] f32, plus acc
            aldr = []
            for r in range(NRANGE):
                aldr_t = persist.tile([128, T, 8], F32, name=f"aldr{r}", tag=f"aldr{r}")
                aldr.append(aldr_t)
            acc = persist.tile([128, T, 72], F32)

            # ---------------- phase G: node feature table + al_d mini-pass
            with (
                tc.tile_pool(name="gpsum", bufs=2, space="PSUM") as gpsum,
                tc.tile_pool(name="gchunk", bufs=3) as gchunk,
                tc.tile_pool(name="gout", bufs=3) as gout,
            ):
                CPG = 6
                for g0 in (range(0, nch, CPG) if "g" in phases else []):
                    gn = min(CPG, nch - g0)
                    ps = gpsum.tile([128, CPG * 80], F32, tag="gps")
                    xc = gchunk.tile([128, CPG, D_IN], BF16, tag="xc")
                    nc.sync.dma_start(
                        xc[:, :gn, :],
                        xt.ap()[g0 * 128 : (g0 + gn) * 128, :].rearrange(
                            "(c d) n -> d c n", d=D_IN
                        ),
                    )
                    for i in range(gn):
                        nc.tensor.matmul(
                            out=ps[:, i * 80 : i * 80 + 80],
                            lhsT=xc[:, i, :],
                            rhs=wcat[:],
                            start=True,
                            stop=True,
                        )
                    gb = gout.tile([128, CPG, GROW], BF16, tag="gb")
                    gbf = gb[:].bitcast(F32)  # [128, CPG, 64]
                    psv = ps[:].rearrange("p (c f) -> p c f", f=80)
                    nc.scalar.copy(gb[:, :gn, 0:64], psv[:, :gn, 0:64])
                    nc.scalar.copy(gbf[:, :gn, 32:40], psv[:, :gn, 64:72])
                    dst = gtab.ap()[g0 * 128 : (g0 + gn) * 128, :].rearrange(
                        "(c p) f -> p c f", p=128
                    )
                    nc.sync.dma_start(dst, gb[:, :gn, :])
                # sentinel rows: al_s = -200
                sent = persist.tile([1, 8], F32)
                nc.vector.memset(sent[:], SENT_ALS)
                for r in (range(NRANGE) if "g" in phases else []):
                    row = r * win + rng_sz
                    nc.sync.dma_start(
                        gtab.ap().bitcast(F32)[row : row + 1, 32:40], sent[:]
                    )
                # al_d mini-pass over per-range permuted local xT
                CPA = 16
                for r in (range(NRANGE) if "a" in phases else []):
                    for t0 in range(0, T, CPA):
                        tn = min(CPA, T - t0)
                        psa = gpsum.tile([128, CPA * 8], F32, tag="psa")
                        xca = gchunk.tile([128, CPA, D_IN], BF16, tag="xca")
                        base = (r * T + t0) * D_IN
                        nc.sync.dma_start(
                            xca[:, :tn, :],
                            xtloc.ap()[base : base + tn * D_IN, :].rearrange(
                                "(c d) n -> d c n", d=D_IN
                            ),
                        )
                        for i in range(tn):
                            nc.tensor.matmul(
                                out=psa[:, i * 8 : i * 8 + 8],
                                lhsT=xca[:, i, :],
                                rhs=wcat[:, 72:80],
                                start=True,
                                stop=True,
                            )
                        nc.scalar.copy(
                            aldr[r][:, t0 : t0 + tn, :],
                            psa[:].rearrange("p (c f) -> p c f", f=8)[:, :tn, :],
                        )

            _elvl = 4
            for _ch in phases:
                if _ch.isdigit():
                    _elvl = int(_ch)
            if "e" not in phases or _elvl < 4:
                nc.vector.memset(acc[:], 1.0)
                for r in range(NRANGE):
                    if "a" not in phases:
                        nc.vector.memset(aldr[r][:], 0.0)
            # ---------------- phase E: edge aggregation, one pass per range
            with (
                tc.tile_pool(name="gath", bufs=3) as gath,
                tc.tile_pool(name="eidx", bufs=3) as eidx,
                tc.tile_pool(name="emsg", bufs=3) as emsg,
                tc.tile_pool(name="ew", bufs=4) as ew,
                tc.tile_pool(name="epsum", bufs=2, space="PSUM") as epsum,
                tc.tile_pool(name="identp", bufs=1) as identp,
            ):
                identb = identp.tile([128, 128], BF16)
                nc.vector.tensor_copy(identb[:], identf[:])
                eqn = 0
                elvl = 4
                for ch in phases:
                    if ch.isdigit():
                        elvl = int(ch)
                for r in (range(NRANGE) if "e" in phases else []):
                    icol = 0
                    for (t0, gn, Kg) in groups[r]:
                        nidx = gn * Kg * 128
                        ncol = nidx // 16
                        idxsb = eidx.tile([128, ncol], I16, tag="idx")
                        nc.sync.dma_start(
                            idxsb[:], idx_d[r].ap()[:, icol : icol + ncol]
                        )
                        icol += ncol
                        gth = gath.tile([128, gn * Kg, GROW], BF16, tag="gth")
                        nc.gpsimd.dma_gather(
                            out_ap=gth[:],
                            in_ap=gtab.ap()[r * win : (r + 1) * win, :],
                            idxs_ap=idxsb[:],
                            num_idxs=nidx,
                            num_idxs_reg=nidx,
                            elem_size=GROW,
                            single_packet=False,
                        )
                        if elvl < 2:
                            continue
                        gthf = gth[:].bitcast(F32)  # [128, gn*Kg, 64]
                        # logits = al_s[src] + al_d[dst] for the whole group
                        w0t = ew.tile([128, gn * Kg, 8], F32, tag="w0")
                        nc.vector.tensor_tensor(
                            out=w0t[:].rearrange("p (g k) f -> p g k f", k=Kg),
                            in0=gthf[:, :, 32:40].rearrange(
                                "p (g k) f -> p g k f", k=Kg
                            ),
                            in1=aldr[r][:, t0 : t0 + gn, None, :].broadcast_to(
                                (128, gn, Kg, 8)
                            ),
                            op=ALU.add,
                        )
                        nc.vector.scalar_tensor_tensor(
                            out=w0t[:],
                            in0=w0t[:],
                            scalar=NEG_SLOPE,
                            in1=w0t[:],
                            op0=ALU.mult,
                            op1=ALU.max,
                        )
                        msg = emsg.tile([128, gn * Kg, 72], BF16, tag="msg")
                        nc.scalar.activation(msg[:, :, 64:72], w0t[:], AF.Exp)
                        if elvl < 3:
                            continue
                        nc.vector.tensor_tensor(
                            out=msg[:, :, 0:64].rearrange(
                                "p k (h j) -> p k h j", h=8
                            ),
                            in0=gth[:, :, 0:64].rearrange(
                                "p k (h j) -> p k h j", h=8
                            ),
                            in1=msg[:, :, 64:72][:, :, :, None].broadcast_to(
                                (128, gn * Kg, 8, 8)
                            ),
                            op=ALU.mult,
                        )
                        if elvl < 4:
                            continue
                        # reduce over k via PE identity-matmul accumulation
                        eps = epsum.tile([128, 504], F32, tag="eps")
                        msgv = msg[:].rearrange("p (g k) f -> p g k f", k=Kg)
                        for k in range(Kg):
                            nc.tensor.matmul(
                                out=eps[:, : gn * 72],
                                lhsT=identb[:],
                                rhs=msgv[:, :, k, :],
                                start=(k == 0),
                                stop=(k == Kg - 1),
                            )
                        if r == 0:
                            nc.scalar.copy(
                                acc[:, t0 : t0 + gn, :],
                                eps[:, : gn * 72].rearrange(
                                    "p (g f) -> p g f", f=72
                                ),
                            )
                        else:
                            sm = ew.tile([128, 7, 72], F32, tag="sm")
                            nc.scalar.copy(
                                sm[:, :gn, :],
                                eps[:, : gn * 72].rearrange(
                                    "p (g f) -> p g f", f=72
                                ),
                            )
                            nc.sync.dma_start(
                                partials[r - 1]
                                .ap()[t0 * 128 : (t0 + gn) * 128, 0:72]
                                .rearrange("(g p) f -> p g f", p=128),
                                sm[:, :gn, :],
                            )

            # ---------------- phase C: combine partials into acc, divide
            with (
                tc.tile_pool(name="cmb", bufs=2) as cmb,
                tc.tile_pool(name="cidx", bufs=2) as cidx,
            ):
                HT = T // 2 + (T % 2)
                for r in (range(1, NRANGE) if "c" in phases else []):
                    for half in range(2):
                        t0 = half * HT
                        tn = min(HT, T - t0)
                        if tn <= 0:
                            continue
                        nidx = tn * 128
                        ncol = (nidx + 15) // 16
                        colbase = (r - 1) * ((NT + 15) // 16) + t0 * 8
                        ridx = cidx.tile([128, ncol], I16, tag="ridx")
                        nc.sync.dma_start(
                            ridx[:], rel_d.ap()[:, colbase : colbase + ncol]
                        )
                        gt = cmb.tile([128, tn * 1, 128], F32, tag="cg")
                        nc.gpsimd.dma_gather(
                            out_ap=gt[:],
                            in_ap=partials[r - 1].ap(),
                            idxs_ap=ridx[:],
                            num_idxs=nidx,
                            num_idxs_reg=nidx,
                            elem_size=128,
                            single_packet=False,
                        )
                        nc.vector.tensor_tensor(
                            out=acc[:, t0 : t0 + tn, :],
                            in0=acc[:, t0 : t0 + tn, :],
                            in1=gt[:, :, 0:72],
                            op=ALU.add,
                        )
                # h0 = acc[:, :, 0:64] / (acc[:, :, 64:72] + 1e-16)
                dinv = cmb.tile([128, T, 8], F32, tag="dinv")
                nc.vector.tensor_scalar_add(dinv[:], acc[:, :, 64:72], 1e-16)
                nc.vector.reciprocal(dinv[:], dinv[:])
                h0 = persist.tile([128, T, HID], F32)
                nc.vector.tensor_tensor(
                    out=h0[:].rearrange("p t (h j) -> p t h j", h=8),
                    in0=acc[:, :, 0:64].rearrange("p t (h j) -> p t h j", h=8),
                    in1=dinv[:][:, :, :, None].broadcast_to((128, T, 8, 8)),
                    op=ALU.mult,
                )

            # ---------------- phase T: LN -> relu -> l2norm -> MLPs -> coords
            with (
                tc.tile_pool(name="tbig", bufs=1) as tbig,
                tc.tile_pool(name="tsml", bufs=1) as tsml,
                tc.tile_pool(name="tpsum", bufs=2, space="PSUM") as tpsum,
                tc.tile_pool(name="ttr", bufs=3) as ttr,
            ):
                def layer_norm(src_ap, dst_ap, P, Tn, D):
                    mn = tsml.tile([P, Tn], F32, tag="ln_mn")
                    nc.vector.tensor_reduce(
                        out=mn[:], in_=src_ap, axis=mybir.AxisListType.X, op=ALU.add
                    )
                    nc.vector.tensor_scalar_mul(mn[:], mn[:], 1.0 / D)
                    nc.vector.tensor_tensor(
                        out=dst_ap,
                        in0=src_ap,
                        in1=mn[:][:, :, None].broadcast_to((P, Tn, D)),
                        op=ALU.subtract,
                    )
                    sq = tbig.tile([P, Tn, D], F32, tag="lnsq")
                    nc.scalar.square(sq[:], dst_ap)
                    var = tsml.tile([P, Tn], F32, tag="ln_var")
                    nc.vector.tensor_reduce(
                        out=var[:], in_=sq[:], axis=mybir.AxisListType.X, op=ALU.add
                    )
                    nc.scalar.activation(
                        var[:], var[:], AF.Sqrt, bias=eps_c[:], scale=1.0 / D
                    )
                    nc.vector.reciprocal(var[:], var[:])
                    nc.vector.tensor_tensor(
                        out=dst_ap,
                        in0=dst_ap,
                        in1=var[:][:, :, None].broadcast_to((P, Tn, D)),
                        op=ALU.mult,
                    )

                def wrap_angle(dst, src, bias):
                    r_ = tsml.tile([128, T], F32, tag="wrap_r")
                    nc.vector.tensor_scalar(
                        out=r_[:],
                        in0=src,
                        scalar1=1.0 / TWO_PI,
                        scalar2=bias / TWO_PI + RND_MAGIC,
                        op0=ALU.mult,
                        op1=ALU.add,
                    )
                    nc.vector.tensor_scalar_add(r_[:], r_[:], -RND_MAGIC)
                    nc.vector.scalar_tensor_tensor(
                        out=r_[:],
                        in0=r_[:],
                        scalar=-TWO_PI,
                        in1=src,
                        op0=ALU.mult,
                        op1=ALU.add,
                    )
                    nc.vector.tensor_scalar_add(dst, r_[:], bias)

                if "t" not in phases:
                    nc.sync.dma_start(coords.ap()[0:128, :], maskt[:, 0:2])
                h1 = tbig.tile([128, T, HID], F32, tag="h1")
                layer_norm(h0[:], h1[:], 128, T, HID)
                nc.scalar.activation(h1[:], h1[:], AF.Relu)
                sq = tbig.tile([128, T, HID], F32, tag="lnsq")
                nc.vector.tensor_tensor(out=sq[:], in0=h1[:], in1=h1[:], op=ALU.mult)
                s2 = tsml.tile([128, T], F32)
                nc.vector.tensor_reduce(
                    out=s2[:], in_=sq[:], axis=mybir.AxisListType.X, op=ALU.add
                )
                nc.vector.tensor_scalar_max(s2[:], s2[:], 1e-24)
                nc.scalar.sqrt(s2[:], s2[:])
                nc.vector.reciprocal(s2[:], s2[:])
                nc.vector.tensor_tensor(
                    out=h1[:],
                    in0=h1[:],
                    in1=s2[:][:, :, None].broadcast_to((128, T, HID)),
                    op=ALU.mult,
                )
                z = tbig.tile([128, T, 96], F32, tag="z")
                for t in range(T):
                    pt = tpsum.tile([HID, 128], F32, tag="tps")
                    nc.tensor.transpose(out=pt[:], in_=h1[:, t, :], identity=identf[:])
                    hnT = ttr.tile([HID, 128], F32, tag="hnT")
                    nc.scalar.copy(hnT[:], pt[:])
                    zps = tpsum.tile([128, 96], F32, tag="zps")
                    nc.tensor.matmul(
                        out=zps[:], lhsT=hnT[:], rhs=awcat[:], start=True, stop=True
                    )
                    nc.scalar.copy(z[:, t, :], zps[:])
                layer_norm(z[:, :, 0:64], z[:, :, 0:64], 128, T, HID)
                layer_norm(z[:, :, 64:96], z[:, :, 64:96], 128, T, 32)
                nc.scalar.activation(z[:], z[:], AF.Relu)
                tmp = tbig.tile([128, T, HID], F32, tag="lnsq")
                nc.vector.tensor_tensor(
                    out=tmp[:],
                    in0=z[:, :, 0:64],
                    in1=aw2r[:][:, None, :].broadcast_to((128, T, HID)),
                    op=ALU.mult,
                )
                ang = tsml.tile([128, T], F32)
                nc.vector.tensor_reduce(
                    out=ang[:], in_=tmp[:], axis=mybir.AxisListType.X, op=ALU.add
                )
                tmp2 = tbig.tile([128, T, 32], F32, tag="tmp2")
                nc.vector.tensor_tensor(
                    out=tmp2[:],
                    in0=z[:, :, 64:96],
                    in1=rw2r[:][:, None, :].broadcast_to((128, T, 32)),
                    op=ALU.mult,
                )
                rad = tsml.tile([128, T], F32)
                nc.vector.tensor_reduce(
                    out=rad[:], in_=tmp2[:], axis=mybir.AxisListType.X, op=ALU.add
                )
                nc.scalar.activation(rad[:], rad[:], AF.Sigmoid)
                nc.scalar.activation(rad[:], rad[:], AF.Identity, bias=b09_c[:], scale=0.2)
                ax = tsml.tile([128, T], F32)
                ay = tsml.tile([128, T], F32)
                wrap_angle(ax[:], ang[:], math.pi / 2)
                wrap_angle(ay[:], ang[:], 0.0)
                cx = tsml.tile([128, T], F32)
                cy = tsml.tile([128, T], F32)
                nc.scalar.activation(cx[:], ax[:], AF.Sin)
                nc.scalar.activation(cy[:], ay[:], AF.Sin)
                crd = tbig.tile([128, T, 2], F32, tag="crd")
                nc.vector.tensor_tensor(out=crd[:, :, 0], in0=cx[:], in1=rad[:], op=ALU.mult)
                nc.vector.tensor_tensor(out=crd[:, :, 1], in0=cy[:], in1=rad[:], op=ALU.mult)
                nc.vector.tensor_tensor(
                    out=crd[:],
                    in0=crd[:],
                    in1=maskt[:][:, :, None].broadcast_to((128, T, 2)),
                    op=ALU.mult,
                )
                ps2 = tsml.tile([128, 2], F32)
                nc.vector.tensor_reduce(
                    out=ps2[:],
                    in_=crd[:].rearrange("p t c -> p c t"),
                    axis=mybir.AxisListType.X,
                    op=ALU.add,
                )
                pr = tsml.tile([128, 2], F32)
                nc.gpsimd.partition_all_reduce(
                    pr[:], ps2[:], channels=128, reduce_op=bass_isa.ReduceOp.add
                )
                nc.sync.dma_start(ar1i.ap(), pr[0:1, :])
                if not timing:
                    ccsem = nc.alloc_semaphore("ccsem")
                    dsem = nc.alloc_semaphore("ccdsem")
                mean_t = tsml.tile([1, 2], F32)
                if timing:
                    nc.vector.memset(mean_t[:], 0.0)
                else:
                    with tc.tile_critical():
                        nc.gpsimd.collective_compute(
                            "AllReduce",
                            ALU.add,
                            replica_groups=rg,
                            ins=[ar1i.ap().opt()],
                            outs=[ar1o.ap().opt()],
                        ).then_inc(ccsem, 1)
                        nc.gpsimd.wait_ge(ccsem, 1)
                        nc.gpsimd.dma_start(out=mean_t[:], in_=ar1o.ap()).then_inc(dsem, 16)
                        nc.gpsimd.wait_ge(dsem, 16)
                nc.vector.tensor_scalar_mul(mean_t[:], mean_t[:], 1.0 / N)
                meanb = tsml.tile([128, 2], F32)
                nc.gpsimd.partition_broadcast(meanb[:], mean_t[:])
                nc.vector.tensor_tensor(
                    out=crd[:],
                    in0=crd[:],
                    in1=meanb[:][:, None, :].broadcast_to((128, T, 2)),
                    op=ALU.subtract,
                )
                sq2 = tbig.tile([128, T, 2], F32, tag="sq2")
                nc.vector.tensor_tensor(out=sq2[:], in0=crd[:], in1=crd[:], op=ALU.mult)
                n2 = tsml.tile([128, T], F32)
                nc.vector.tensor_reduce(
                    out=n2[:], in_=sq2[:], axis=mybir.AxisListType.X, op=ALU.add
                )
                nm = tsml.tile([128, T], F32)
                nc.vector.scalar_tensor_tensor(
                    out=nm[:],
                    in0=n2[:],
                    scalar=1.0,
                    in1=maskt[:],
                    op0=ALU.add,
                    op1=ALU.mult,
                )
                mx = tsml.tile([128, 1], F32)
                nc.vector.tensor_reduce(
                    out=mx[:], in_=nm[:], axis=mybir.AxisListType.X, op=ALU.max
                )
                mxr = tsml.tile([128, 1], F32)
                nc.gpsimd.partition_all_reduce(
                    mxr[:], mx[:], channels=128, reduce_op=bass_isa.ReduceOp.max
                )
                nc.sync.dma_start(ar2i.ap(), mxr[0:1, :])
                scl_t = tsml.tile([1, 1], F32)
                if timing:
                    nc.vector.memset(scl_t[:], 2.0)
                else:
                    with tc.tile_critical():
                        nc.gpsimd.collective_compute(
                            "AllReduce",
                            ALU.max,
                            replica_groups=rg,
                            ins=[ar2i.ap().opt()],
                            outs=[ar2o.ap().opt()],
                        ).then_inc(ccsem, 1)
                        nc.gpsimd.wait_ge(ccsem, 2)
                        nc.gpsimd.dma_start(out=scl_t[:], in_=ar2o.ap()).then_inc(dsem, 16)
                        nc.gpsimd.wait_ge(dsem, 32)
                nc.vector.tensor_scalar_add(scl_t[:], scl_t[:], -1.0)
                nc.scalar.sqrt(scl_t[:], scl_t[:])
                nc.vector.tensor_scalar_add(scl_t[:], scl_t[:], 1e-8)
                nc.vector.reciprocal(scl_t[:], scl_t[:])
                sclb = tsml.tile([128, 1], F32)
                nc.gpsimd.partition_broadcast(sclb[:], scl_t[:])
                nc.vector.tensor_scalar_mul(crd[:], crd[:], sclb[:, 0:1])
                nc.sync.dma_start(
                    coords.ap().rearrange("(t p) c -> p t c", p=128), crd[:]
                )

    nc.compile()
    return nc


# ---------------------------------------------------------------- runner
def _run(x, edge_index, params, N, NC, reduce_mode="dve", trace=False):
    prep = _host_prep(x, edge_index, N, NC)
    npc, T = prep["npc"], prep["T"]

    p = {k: np.asarray(v, np.float32) for k, v in params.items()}
    for z in ["bias0", "lnb0", "ab1", "ab2", "rb1", "rb2", "alb", "rlb"]:
        assert np.abs(p[z]).max() == 0.0, z
    for o in ["lng0", "alg", "rlg"]:
        assert np.abs(p[o] - 1.0).max() == 0.0, o

    A_s = np.zeros((HID, HEADS), np.float32)
    A_d = np.zeros((HID, HEADS), np.float32)
    for h in range(HEADS):
        A_s[h * HD : (h + 1) * HD, h] = p["as0"][h]
        A_d[h * HD : (h + 1) * HD, h] = p["ad0"][h]
    wcat = np.ascontiguousarray(
        np.concatenate([p["W0"], p["W0"] @ A_s, p["W0"] @ A_d], axis=1), np.float32
    )
    awcat = np.ascontiguousarray(np.concatenate([p["aw1"], p["rw1"]], axis=1))
    aw2r = np.ascontiguousarray(np.tile(p["aw2"].reshape(1, HID), (128, 1)))
    rw2r = np.ascontiguousarray(np.tile(p["rw2"].reshape(1, 32), (128, 1)))

    nc = _build_program(
        N, NC, prep["Kt"], prep["groups"], prep["kofs"], reduce_mode=reduce_mode
    )

    in_maps = []
    for c in range(NC):
        m = dict(
            xt=prep["xtc"],
            xtloc=prep["xtloc"][c],
            wcat=wcat,
            rel=prep["rel"][c],
            maskt=prep["maskt"],
            awcat=awcat,
            aw2r=aw2r,
            rw2r=rw2r,
        )
        for r in range(NRANGE):
            m[f"idx{r}"] = prep["idx"][c][r]
        in_maps.append(m)
    res = run_bass_kernel_spmd(nc, in_maps, list(range(NC)), trace=trace)
    out = np.empty((N, 2), np.float32)
    for c in range(NC):
        out[prep["perms"][c][0]] = res.results[c]["coords"][:npc]
    return out, res


def kernel(x, edge_index, params):
    out, _ = _run(x, edge_index, params, FULL["N"], FULL["NC"])
    return out
